# revision 19
# baseline (speedup 1.0000x reference)
"""Trainium2 Bass kernel for a dense transformer block (B=8, N=1024, C=768, H=12).

Sharding: pure data-parallel over batch — core b computes batch element b.
No collectives. Host prepares per-core inputs (transposed k_conn, folded /
transposed weights in fp16) and reassembles the [8, 1024, 768] output.

Layout strategy per core:
  - LN1/LN2 statistics run token-major (free-dim reductions); normalization is
    applied in feature-major space with DRAM-bounced, partition-broadcast
    mean/rstd rows, so LN1 needs no transposes (host supplies xT) and only
    LN2's zn needs PE transposes.
  - Attention computes scores TRANSPOSED (scoresT[m, n] = k·q) so the softmax
    denominator (a partition-dim reduction) folds into the attn@v matmul via a
    ones-column augmentation of V (host-interleaved); no attention transposes.
  - softmax skips max-subtraction (scores bounded ~±6; exp is safe in fp32).
  - The softmax reciprocal is reshaped through DRAM onto 128 partitions
    (single-partition DVE reciprocal is ~8 cyc/elem on one lane).
"""

import os
import sys

import numpy as np

for _p in ("/opt/trn_rl_repo", "/root/.axon_site/_ro/trn_rl_repo"):
    if os.path.isdir(_p) and _p not in sys.path:
        sys.path.insert(0, _p)

import concourse.bass as bass
import concourse.bacc as bacc
import concourse.tile as tile
from concourse import mybir
from concourse.bass_utils import run_bass_kernel_spmd
from concourse.masks import make_identity

B, N, C, H = 8, 1024, 768, 12
HS = C // H                 # 64 head size
SCALE = HS ** -0.5
EPS = 1e-5
P = 128                     # partitions
NT = N // P                 # 8 token tiles
CC = C // P                 # 6 channel chunks
DT = (2 * C) // P           # 12 M-tiles covering q then k
VW = H * (HS + 1)           # 780: v columns with a ones-column per head
AF = mybir.ActivationFunctionType
f32 = mybir.dt.float32
f16 = mybir.dt.float16


def _ln_stats(nc, tp, x_ap, eps_t):
    """LN stats of a [128, 768] fp32 tile -> (mv fp32 [P,2], rstd fp32 [P,1])."""
    stats = tp.tile([P, 3, nc.vector.BN_STATS_DIM], f32, tag="ln_stats", bufs=2)
    for s in range(3):
        nc.vector.bn_stats(out=stats[:, s, :], in_=x_ap[:, s * 256:(s + 1) * 256])
    mv = tp.tile([P, nc.vector.BN_AGGR_DIM], f32, tag="ln_mv", bufs=2)
    nc.vector.bn_aggr(out=mv, in_=stats)
    std = tp.tile([P, 1], f32, tag="ln_std", bufs=2)
    nc.scalar.activation(out=std, in_=mv[:, 1:2], func=AF.Sqrt,
                         bias=eps_t[:, 0:1], scale=1.0)
    rstd = tp.tile([P, 1], f32, tag="ln_rstd", bufs=2)
    nc.vector.reciprocal(out=rstd, in_=std)
    return mv, rstd


def _bounce_stat_rows(nc, tp, i, mv, rstd, mu_d, rs_d):
    """Convert per-token mean/rstd columns to f16 and park them in DRAM rows."""
    mu16 = tp.tile([P, 1], f16, tag="mu16", bufs=2)
    nc.vector.tensor_copy(mu16[:], mv[:, 0:1])
    rs16 = tp.tile([P, 1], f16, tag="rs16", bufs=2)
    nc.vector.tensor_copy(rs16[:], rstd[:])
    nc.sync.dma_start(out=mu_d[i * P:(i + 1) * P][:, None], in_=mu16[:])
    nc.sync.dma_start(out=rs_d[i * P:(i + 1) * P][:, None], in_=rs16[:])


def _bcast_sb_row(nc, pool, psp, row_ap, ones_row, tag, rows=P, bufs=1):
    """Broadcast an SBUF [1, N] f16 row to [rows, N] f16 via K=1 ones-matmul."""
    bps = psp.tile([rows, N], f32, tag="ps", name=tag + "_ps")
    for nj in range(2):
        nc.tensor.matmul(bps[:, nj * 512:(nj + 1) * 512],
                         lhsT=ones_row[:, 0:rows],
                         rhs=row_ap[:, nj * 512:(nj + 1) * 512],
                         start=True, stop=True)
    t = pool.tile([rows, N], f16, tag=tag, bufs=bufs, name=tag)
    nc.scalar.copy(out=t[:], in_=bps[:])
    return t


def _bcast_row_pe(nc, pool, psp, src_d, ones_row, tag, rows=P, bufs=1):
    """Broadcast a DRAM f16 row [N] to an SBUF [rows, N] f16 tile via a K=1
    ones-matmul (a partition-broadcast DMA costs one descriptor per
    partition and is ~100x slower)."""
    row = pool.tile([1, N], f16, tag=tag + "_r", bufs=bufs, name=tag + "_r")
    nc.sync.dma_start(out=row[:], in_=src_d[None, :])
    bps = psp.tile([rows, N], f32, tag="ps", name=tag + "_ps")
    for nj in range(2):
        nc.tensor.matmul(bps[:, nj * 512:(nj + 1) * 512],
                         lhsT=ones_row[:, 0:rows],
                         rhs=row[:, nj * 512:(nj + 1) * 512],
                         start=True, stop=True)
    t = pool.tile([rows, N], f16, tag=tag, bufs=bufs, name=tag)
    nc.scalar.copy(out=t[:], in_=bps[:])
    return t


def build_kernel():
    nc = bacc.Bacc("TRN2", target_bir_lowering=False, debug=False,
                   enable_asserts=False)

    x_d = nc.declare_dram_parameter("x", [N, C], f32, isOutput=False)
    xT_d = nc.declare_dram_parameter("xT", [C, N], f16, isOutput=False)
    kcT_d = nc.declare_dram_parameter("kcT", [N, N], f16, isOutput=False)
    wqk_d = nc.declare_dram_parameter("wqkT", [C, 2 * C], f16, isOutput=False)
    bqk_d = nc.declare_dram_parameter("bqk", [2 * C], f16, isOutput=False)
    wv_d = nc.declare_dram_parameter("wvT", [C, VW], f16, isOutput=False)
    bv_d = nc.declare_dram_parameter("bv", [VW], f16, isOutput=False)
    wp_d = nc.declare_dram_parameter("projT", [C, C], f16, isOutput=False)
    bp_d = nc.declare_dram_parameter("proj_b", [C], f16, isOutput=False)
    w1_d = nc.declare_dram_parameter("fc1T", [C, C], f16, isOutput=False)
    b1_d = nc.declare_dram_parameter("fc1_b", [C], f32, isOutput=False)
    w2_d = nc.declare_dram_parameter("fc2T", [C, C], f16, isOutput=False)
    b2_d = nc.declare_dram_parameter("fc2_b", [C], f16, isOutput=False)
    out_d = nc.declare_dram_parameter("out", [N, C], f32, isOutput=True)

    sums_d = nc.dram_tensor("sums_scratch", [H, N], f16)
    recq_d = nc.dram_tensor("recq_scratch", [H, N], f16)
    mu1_d = nc.dram_tensor("mu1_row", [N], f16)
    rs1_d = nc.dram_tensor("rs1_row", [N], f16)

    with tile.TileContext(nc) as tc:
        with (
            tc.tile_pool(name="consts", bufs=1) as consts,
            tc.tile_pool(name="acts", bufs=1) as acts,
            tc.tile_pool(name="tp", bufs=3) as tp,
            tc.tile_pool(name="ps", bufs=2, space="PSUM") as psp,
            tc.tile_pool(name="po", bufs=2, space="PSUM") as pop,
        ):
            # ---------------- constants / weights ----------------
            eps_t = consts.tile([P, 1], f32)
            nc.vector.memset(eps_t, EPS)
            ones_row = consts.tile([1, 512], f16)
            nc.vector.memset(ones_row, 1.0)
            ident = consts.tile([P, P], f16)
            make_identity(nc, ident[:])

            def load_chunked(dst, src_re, n_chunk):
                for c in range(n_chunk):
                    nc.sync.dma_start(out=dst[:, c], in_=src_re[:, c])

            # ---------------- phase A: LN1 -> xnT (feature-major) -----------
            # Stats computed on the PE from host-transposed xT (f16):
            # mean and E[x^2] via (1/C)-ones matmuls over the partition dim,
            # rstd = exp(-0.5 ln(var+eps)) on ACT. No token-major x load and
            # no per-tile DRAM stat bounces in the critical prologue.
            xT_sb = acts.tile([P, CC, N], f16, tag="attn_oT")  # host-transposed x
            load_chunked(xT_sb, xT_d.rearrange("(ci p) n -> p ci n", p=P), CC)
            oc = consts.tile([P, 1], f16)
            nc.vector.memset(oc, 1.0 / C)
            mu_ps = psp.tile([1, N], f32, tag="ps", name="mu_ps")
            esq_ps = psp.tile([1, N], f32, tag="ps", name="esq_ps")
            for ci in range(CC):
                sq = tp.tile([P, N], f16, tag="cen", bufs=2, name="sq")
                nc.scalar.activation(out=sq[:], in_=xT_sb[:, ci, :], func=AF.Square)
                for nj in range(2):
                    nc.tensor.matmul(mu_ps[:, nj * 512:(nj + 1) * 512],
                                     lhsT=oc[:], rhs=xT_sb[:, ci,
                                                          nj * 512:(nj + 1) * 512],
                                     start=(ci == 0), stop=(ci == CC - 1))
                    nc.tensor.matmul(esq_ps[:, nj * 512:(nj + 1) * 512],
                                     lhsT=oc[:], rhs=sq[:,
                                                       nj * 512:(nj + 1) * 512],
                                     start=(ci == 0), stop=(ci == CC - 1))
            mu_sb = tp.tile([1, N], f32, tag="row32", bufs=2, name="mu_sb")
            nc.scalar.copy(out=mu_sb[:], in_=mu_ps[:])
            musq = tp.tile([1, N], f32, tag="row32", bufs=2, name="musq")
            nc.vector.tensor_mul(musq[:], mu_sb[:], mu_sb[:])
            var_sb = tp.tile([1, N], f16, tag="rowf16", bufs=4, name="var_sb")
            nc.vector.tensor_tensor(out=var_sb[:], in0=esq_ps[:], in1=musq[:],
                                    op=mybir.AluOpType.subtract)
            lnv = tp.tile([1, N], f16, tag="rowf16", bufs=4, name="lnv")
            nc.scalar.activation(out=lnv[:], in_=var_sb[:], func=AF.Ln,
                                 bias=eps_t[0:1, 0:1])
            rstd_row = tp.tile([1, N], f16, tag="rowf16", bufs=4, name="rstd_row")
            nc.scalar.activation(out=rstd_row[:], in_=lnv[:], func=AF.Exp,
                                 scale=-0.5)
            mu_row = tp.tile([1, N], f16, tag="rowf16", bufs=4, name="mu_row")
            nc.vector.tensor_copy(mu_row[:], mu_sb[:])
            mu1_b = _bcast_sb_row(nc, tp, psp, mu_row[:], ones_row, "mu_b")
            rs1_b = _bcast_sb_row(nc, tp, psp, rstd_row[:], ones_row, "rs_b")
            xnT = acts.tile([P, CC, N], f16, tag="fm_act")  # feature-major LN1 out
            for ci in range(CC):
                cen = tp.tile([P, N], f16, tag="cen", bufs=2)
                nc.vector.tensor_tensor(out=cen[:], in0=xT_sb[:, ci, :],
                                        in1=mu1_b[:], op=mybir.AluOpType.subtract)
                nc.vector.tensor_tensor(out=xnT[:, ci, :], in0=cen[:],
                                        in1=rs1_b[:], op=mybir.AluOpType.mult)

            wqk_sb = consts.tile([P, CC, 2 * C], f16)
            load_chunked(wqk_sb, wqk_d.rearrange("(ci p) d -> p ci d", p=P), CC)
            bqk_row = consts.tile([1, 2 * C], f16)
            nc.sync.dma_start(out=bqk_row, in_=bqk_d[None, :])
            wv_sb = consts.tile([P, CC, VW], f16)
            load_chunked(wv_sb, wv_d.rearrange("(ci p) d -> p ci d", p=P), CC)
            bv_row = consts.tile([1, VW], f16)
            nc.sync.dma_start(out=bv_row, in_=bv_d[None, :])
            kcT_sb = acts.tile([P, NT, N], f16, tag="kcT")
            load_chunked(kcT_sb, kcT_d.rearrange("(mi p) n -> p mi n", p=P), NT)

            # ---------------- phase B: q^T, k^T ----------------
            qkT = acts.tile([P, DT, N], f16, tag="qkT")
            for t in range(DT):
                ps = psp.tile([P, N], f32, tag="ps", name="ps_qk")
                for nj in range(2):
                    for ci in range(CC):
                        nc.tensor.matmul(
                            ps[:, nj * 512:(nj + 1) * 512],
                            lhsT=wqk_sb[:, ci, t * P:(t + 1) * P],
                            rhs=xnT[:, ci, nj * 512:(nj + 1) * 512],
                            start=(ci == 0), stop=False)
                    nc.tensor.matmul(ps[:, nj * 512:(nj + 1) * 512],
                                     lhsT=bqk_row[:, t * P:(t + 1) * P],
                                     rhs=ones_row[:, 0:512],
                                     start=False, stop=True)
                nc.scalar.copy(out=qkT[:, t, :], in_=ps[:])

            # ---------------- phase C: V (token-major, ones-augmented) -------
            v_aug = acts.tile([P, NT, VW], f16, tag="v_aug")
            for mi in range(NT):
                ps = psp.tile([P, VW], f32, tag="ps")
                for c0, c1 in ((0, 512), (512, VW)):
                    for ci in range(CC):
                        nc.tensor.matmul(
                            ps[:, c0:c1],
                            lhsT=xnT[:, ci, mi * P:(mi + 1) * P],
                            rhs=wv_sb[:, ci, c0:c1],
                            start=(ci == 0), stop=False)
                    nc.tensor.matmul(ps[:, c0:c1], lhsT=ones_row[:, 0:P],
                                     rhs=bv_row[:, c0:c1], start=False, stop=True)
                nc.scalar.copy(out=v_aug[:, mi, :], in_=ps[:])

            # ---------------- phase D: attention per head ----------------
            attn_oT = acts.tile([P, CC, N], f16, tag="attn_oT")
            NS = NT // 2                      # 4 slabs of 2 token tiles
            for h in range(H):
                t_q, off = h // 2, (h % 2) * HS
                t_k = CC + h // 2
                po = pop.tile([HS + 1, N], f32, tag="po")
                exp_sl = [None] * NS

                def scores_slab(s):
                    ms = tp.tile([P, 2, N], f16, tag="ms", bufs=2, name="ms")
                    for q in range(2):
                        mi = 2 * s + q
                        ps = psp.tile([P, N], f32, tag="ps", name="ps")
                        for nj in range(2):
                            nc.tensor.matmul(
                                ps[:, nj * 512:(nj + 1) * 512],
                                lhsT=qkT[off:off + HS, t_k, mi * P:(mi + 1) * P],
                                rhs=qkT[off:off + HS, t_q, nj * 512:(nj + 1) * 512],
                                start=True, stop=True)
                        nc.vector.tensor_mul(ms[:, q, :], ps[:], kcT_sb[:, mi, :])
                    expT = tp.tile([P, 2, N], f16, tag="expT", bufs=3, name="expT")
                    nc.scalar.activation(out=expT[:], in_=ms[:], func=AF.Exp)
                    exp_sl[s] = expT

                def attnv_slab(s):
                    for q in range(2):
                        mi = 2 * s + q
                        for nj in range(2):
                            nc.tensor.matmul(
                                po[:, nj * 512:(nj + 1) * 512],
                                lhsT=v_aug[:, mi, h * (HS + 1):(h + 1) * (HS + 1)],
                                rhs=exp_sl[s][:, q, nj * 512:(nj + 1) * 512],
                                start=(mi == 0), stop=(mi == NT - 1))

                # stagger attn@v one slab behind scores so the PE never waits
                # on the exp of the slab it is about to consume
                scores_slab(0)
                scores_slab(1)
                attnv_slab(0)
                scores_slab(2)
                attnv_slab(1)
                scores_slab(3)
                attnv_slab(2)
                attnv_slab(3)

                # evacuate PSUM: unnormalized head output straight into
                # attn_oT; sums row bounced to DRAM for the reshaped recip
                nc.scalar.copy(out=attn_oT[off:off + HS, h // 2, :],
                               in_=po[0:HS, :])
                sums_sb = tp.tile([1, N], f16, tag="sums_sb", bufs=2)
                nc.scalar.copy(out=sums_sb[:], in_=po[HS:HS + 1, :])
                nc.sync.dma_start(out=sums_d[h, :][None, :], in_=sums_sb[:])
                srows = tp.tile([P, NT], f16, tag="srows", bufs=2)
                nc.sync.dma_start(
                    out=srows[:],
                    in_=sums_d[h, :].rearrange("(p a) -> p a", p=P))
                rec = tp.tile([P, NT], f16, tag="rec", bufs=2)
                with nc.allow_low_precision(reason="attn weights are f16 anyway"):
                    nc.vector.reciprocal(out=rec[:], in_=srows[:])
                nc.sync.dma_start(
                    out=recq_d[h, :].rearrange("(p a) -> p a", p=P), in_=rec[:])

            # normalize all heads at once: rb_c[p, n] = 1/sums[head(p), n],
            # built per channel-chunk with two K=1 ones-matmul broadcasts
            for ci in range(CC):
                ra = tp.tile([1, N], f16, tag="ra", bufs=1)
                nc.sync.dma_start(out=ra[:], in_=recq_d[2 * ci, :][None, :])
                rb = tp.tile([1, N], f16, tag="rbrow", bufs=1)
                nc.sync.dma_start(out=rb[:], in_=recq_d[2 * ci + 1, :][None, :])
                rb_ps = psp.tile([P, N], f32, tag="ps", name="rb_ps")
                for nj in range(2):
                    nc.tensor.matmul(rb_ps[0:HS, nj * 512:(nj + 1) * 512],
                                     lhsT=ones_row[:, 0:HS],
                                     rhs=ra[:, nj * 512:(nj + 1) * 512],
                                     start=True, stop=True)
                    nc.tensor.matmul(rb_ps[HS:P, nj * 512:(nj + 1) * 512],
                                     lhsT=ones_row[:, 0:HS],
                                     rhs=rb[:, nj * 512:(nj + 1) * 512],
                                     start=True, stop=True)
                rb_c = tp.tile([P, N], f16, tag="rb_c", bufs=1)
                nc.scalar.copy(out=rb_c[:], in_=rb_ps[:])
                nc.vector.tensor_mul(attn_oT[:, ci, :], attn_oT[:, ci, :],
                                     rb_c[:])

            wp_sb = consts.tile([P, CC, C], f16)
            load_chunked(wp_sb, wp_d.rearrange("(ci p) d -> p ci d", p=P), CC)
            bp_row = consts.tile([1, C], f16)
            nc.sync.dma_start(out=bp_row, in_=bp_d[None, :])
            w1_sb = consts.tile([P, CC, C], f16)
            load_chunked(w1_sb, w1_d.rearrange("(ci p) d -> p ci d", p=P), CC)
            b1_sb = consts.tile([P, CC], f32)
            nc.sync.dma_start(out=b1_sb, in_=b1_d.rearrange("(t p) -> p t", p=P))
            w2_sb = consts.tile([P, CC, C], f16)
            load_chunked(w2_sb, w2_d.rearrange("(ci p) d -> p ci d", p=P), CC)
            b2_row = consts.tile([1, C], f16)
            nc.sync.dma_start(out=b2_row, in_=b2_d[None, :])

            # ---------------- phase E: proj + residual + LN2 -> znT ----------
            y_sb = acts.tile([P, NT, C], f32, tag="qkT")
            zn_all = acts.tile([P, NT, C], f16, tag="v_aug")
            znT = acts.tile([P, CC, N], f16, tag="fm_act")
            for ni in range(NT):
                ps = psp.tile([P, C], f32, tag="ps")
                for c0, c1 in ((0, 512), (512, C)):
                    for ci in range(CC):
                        nc.tensor.matmul(
                            ps[:, c0:c1],
                            lhsT=attn_oT[:, ci, ni * P:(ni + 1) * P],
                            rhs=wp_sb[:, ci, c0:c1],
                            start=(ci == 0), stop=False)
                    nc.tensor.matmul(ps[:, c0:c1], lhsT=ones_row[:, 0:P],
                                     rhs=bp_row[:, c0:c1], start=False, stop=True)
                x_t = tp.tile([P, C], f32, tag="x_in", bufs=2)
                nc.sync.dma_start(out=x_t, in_=x_d[ni * P:(ni + 1) * P, :])
                nc.vector.tensor_add(y_sb[:, ni, :], x_t[:], ps[:])
                mv, rstd = _ln_stats(nc, tp, y_sb[:, ni, :], eps_t)
                nc.gpsimd.tensor_scalar(out=zn_all[:, ni, :], in0=y_sb[:, ni, :],
                                        scalar1=mv[:, 0:1], scalar2=rstd[:],
                                        op0=mybir.AluOpType.subtract,
                                        op1=mybir.AluOpType.mult)
            for ci in range(CC):
                for ni in range(NT):
                    pt = psp.tile([P, P], f16, tag="ps", name="pt")
                    nc.tensor.transpose(pt[:], zn_all[:, ni, ci * P:(ci + 1) * P],
                                        ident[:])
                    nc.scalar.copy(out=znT[:, ci, ni * P:(ni + 1) * P], in_=pt[:])

            # ---------------- phase F: fc1 + exact gelu -> hgT ----------------
            hgT = acts.tile([P, CC, N], f16, tag="attn_oT")
            for t in range(CC):
                ps = psp.tile([P, N], f32, tag="ps")
                for nj in range(2):
                    for ci in range(CC):
                        nc.tensor.matmul(
                            ps[:, nj * 512:(nj + 1) * 512],
                            lhsT=w1_sb[:, ci, t * P:(t + 1) * P],
                            rhs=znT[:, ci, nj * 512:(nj + 1) * 512],
                            start=(ci == 0), stop=(ci == CC - 1))
                nc.scalar.activation(out=hgT[:, t, :], in_=ps[:],
                                     func=AF.Gelu, bias=b1_sb[:, t:t + 1])

            # ---------------- phase G: fc2 + residual -> out ----------------
            for ni in range(NT):
                ps = psp.tile([P, C], f32, tag="ps")
                for c0, c1 in ((0, 512), (512, C)):
                    for ci in range(CC):
                        nc.tensor.matmul(
                            ps[:, c0:c1],
                            lhsT=hgT[:, ci, ni * P:(ni + 1) * P],
                            rhs=w2_sb[:, ci, c0:c1],
                            start=(ci == 0), stop=False)
                    nc.tensor.matmul(ps[:, c0:c1], lhsT=ones_row[:, 0:P],
                                     rhs=b2_row[:, c0:c1], start=False, stop=True)
                o_t = tp.tile([P, C], f32, tag="o_out", bufs=2)
                nc.vector.tensor_add(o_t[:], y_sb[:, ni, :], ps[:])
                nc.sync.dma_start(out=out_d[ni * P:(ni + 1) * P, :], in_=o_t[:])

    nc.compile()
    return nc


_NC = None
LAST_RESULTS = None
TRACE = False


def _prep_weights(inputs):
    qkv_w = np.asarray(inputs["qkv_w"], np.float64)
    proj_w = np.asarray(inputs["proj_w"], np.float64)
    fc1_w = np.asarray(inputs["fc1_w"], np.float64)
    fc2_w = np.asarray(inputs["fc2_w"], np.float64)
    ln1_w = np.asarray(inputs["ln1_w"], np.float64)
    ln1_b = np.asarray(inputs["ln1_b"], np.float64)
    ln2_w = np.asarray(inputs["ln2_w"], np.float64)
    ln2_b = np.asarray(inputs["ln2_b"], np.float64)

    wqkvT = (qkv_w * ln1_w[None, :]).T.copy()       # [c, 3C], rows scaled by ln1_w
    qkv_b = ln1_b @ qkv_w.T                          # [3C]
    wqkT = wqkvT[:, :2 * C].copy()
    wqkT[:, :C] *= SCALE
    bqk = qkv_b[:2 * C].copy()
    bqk[:C] *= SCALE

    wv = wqkvT[:, 2 * C:]                            # [c, C]
    bv = qkv_b[2 * C:]
    wv_aug = np.zeros((C, VW), np.float64)
    bv_aug = np.zeros((VW,), np.float64)
    for h in range(H):
        wv_aug[:, h * (HS + 1):h * (HS + 1) + HS] = wv[:, h * HS:(h + 1) * HS]
        bv_aug[h * (HS + 1):h * (HS + 1) + HS] = bv[h * HS:(h + 1) * HS]
        bv_aug[h * (HS + 1) + HS] = 1.0

    fc1T = (fc1_w * ln2_w[None, :]).T.copy()
    fc1_b_eff = ln2_b @ fc1_w.T + np.asarray(inputs["fc1_b"], np.float64)

    return {
        "wqkT": wqkT.astype(np.float16),
        "bqk": bqk.astype(np.float16),
        "wvT": wv_aug.astype(np.float16),
        "bv": bv_aug.astype(np.float16),
        "projT": proj_w.T.astype(np.float16).copy(),
        "proj_b": np.asarray(inputs["proj_b"], np.float32).astype(np.float16),
        "fc1T": fc1T.astype(np.float16),
        "fc1_b": fc1_b_eff.astype(np.float32),
        "fc2T": fc2_w.T.astype(np.float16).copy(),
        "fc2_b": np.asarray(inputs["fc2_b"], np.float32).astype(np.float16),
    }


def kernel(**inputs):
    global _NC, LAST_RESULTS
    if _NC is None:
        _NC = build_kernel()

    jf = np.ascontiguousarray(np.asarray(inputs["joint_feature"], np.float32))
    kc = np.asarray(inputs["k_conn"], np.float32)
    shared = _prep_weights(inputs)

    in_maps = []
    for b in range(B):
        m = dict(shared)
        m["x"] = jf[b]
        m["xT"] = np.ascontiguousarray(jf[b].T).astype(np.float16)
        m["kcT"] = np.ascontiguousarray(kc[b].T).astype(np.float16)
        in_maps.append(m)

    res = run_bass_kernel_spmd(_NC, in_maps, core_ids=list(range(B)), trace=TRACE)
    LAST_RESULTS = res
    out = np.stack([res.results[b]["out"] for b in range(B)], axis=0)
    return out.astype(np.float32)


if __name__ == "__main__":
    nc = build_kernel()
    print("kernel built OK")
